# revision 1
# baseline (speedup 1.0000x reference)
"""Trainium2 Bass kernel for CrossAttention (B=2, T=S=2048, C=1024, H=16, D=64).

Sharding: 8 cores = 2 batches x 4 head-groups (tensor-parallel over heads,
4 heads per core). Each core computes its heads' attention plus the partial
output projection (Wo row slice); the host sums the 4 partials per batch.

Per-core layout trick: q/p are transposed on the host (qT/pT: [C, T]) so the
contraction dim always lands on SBUF partitions -- no on-chip transposes.

Per-core dataflow (Dc = 256 = 4 heads x 64):
  qhT[Dc,T] = Wq_s.T @ qT     (bf16 in, fp32 accum; stored f32r)
  khT[Dc,S] = Wk_s.T @ pT     (same)
  V[S,Dc]   = pT.T @ Wv_s     (same), stored bf16 with a ones column per
              head (V_aug[s, h, 64] = 1) so the AV matmul also produces the
              softmax row sums (flash-attention style).
  per head h, t-chunk (1024), s-tile (128):
      S^T[s,t] = khT_h.T @ qhT_h          (fp32r, K=64, into PSUM)
      P^T      = exp(0.125 * S^T)         (ACT, PSUM->SBUF, bf16)
      O_aug   += V_aug_h.T @ P^T          (bf16, PSUM accumulation over s)
  normalize: r = 1/rowsum (DVE), broadcast over 64 partitions (gpsimd),
             yT = O * R (DVE, written as f32r)
  out[T, C] = yT.T @ Wo_s  (fp32r) -> DMA; host sums over head groups.
"""

import numpy as np

B, T, S, C, H = 2, 2048, 2048, 1024, 16
D = C // H            # 64
NCORES = 8
HG = 4                # head groups (cores per batch)
HPC = H // HG         # heads per core = 4
DC = HPC * D          # 256 per-core head dims
KT = C // 128         # 8 contraction tiles for projections
NB = T // 512         # 4 free-dim blocks of 512
ST = S // 128         # 16 s-tiles
SCALE = 1.0 / np.sqrt(D)

_COMPILED = None


def _build():
    import concourse.bacc as bacc
    import concourse.tile as tile
    from concourse import mybir

    F32 = mybir.dt.float32
    F32R = mybir.dt.float32r
    BF16 = mybir.dt.bfloat16
    EXP = mybir.ActivationFunctionType.Exp

    nc = bacc.Bacc("TRN2", target_bir_lowering=False, debug=False,
                   num_devices=NCORES)
    qT = nc.dram_tensor("qT", [C, T], BF16, kind="ExternalInput").ap()
    pT = nc.dram_tensor("pT", [C, T], BF16, kind="ExternalInput").ap()
    wq = nc.dram_tensor("wq", [C, DC], BF16, kind="ExternalInput").ap()
    wk = nc.dram_tensor("wk", [C, DC], BF16, kind="ExternalInput").ap()
    wv = nc.dram_tensor("wv", [C, DC], BF16, kind="ExternalInput").ap()
    wo = nc.dram_tensor("wo", [DC, C], F32, kind="ExternalInput").ap()
    out = nc.dram_tensor("out", [T, C], F32, kind="ExternalOutput").ap()

    with tile.TileContext(nc) as tc:
        with (
            tc.tile_pool(name="qx", bufs=32) as qx_pool,         # qT chunks
            tc.tile_pool(name="px", bufs=32) as px_pool,         # pT chunks
            tc.tile_pool(name="w", bufs=1) as w_pool,            # weights
            tc.tile_pool(name="proj", bufs=1) as proj_pool,      # qhT/khT/yT
            tc.tile_pool(name="vaug", bufs=1) as v_pool,
            tc.tile_pool(name="pt", bufs=8) as pt_pool,          # P^T tiles
            tc.tile_pool(name="norm", bufs=2) as norm_pool,
            tc.tile_pool(name="ostage", bufs=3) as o_pool,       # out staging
            tc.tile_pool(name="ps_pp", bufs=4, space="PSUM") as ps_pp,   # 4 banks
            tc.tile_pool(name="ps_sc", bufs=2, space="PSUM") as ps_sc,   # 4 banks
        ):
            # ---- load weights (HWDGE; f32r view of the f32 bits) ----
            wq_sb = w_pool.tile([128, KT, DC], BF16, tag="wq")
            wk_sb = w_pool.tile([128, KT, DC], BF16, tag="wk")
            wv_sb = w_pool.tile([128, KT, DC], BF16, tag="wv")
            wo_sb = w_pool.tile([128, 2, C], F32R, tag="wo")

            def emit_w_dma(w_dram, w_sb):
                nc.sync.dma_start(
                    out=w_sb[:],
                    in_=w_dram.rearrange("(k p) n -> p k n", p=128))

            # qhT/khT/yT: [2 x 128 d, T]; heads 2m,2m+1 in tile m at part 0/64
            qhT = [proj_pool.tile([128, T], F32R, tag=f"qhT{m}", name=f"qhT{m}")
                   for m in range(2)]
            khT = [proj_pool.tile([128, T], F32R, tag=f"khT{m}", name=f"khT{m}")
                   for m in range(2)]
            yT = [proj_pool.tile([128, T], F32R, tag=f"yT{m}", name=f"yT{m}")
                  for m in range(2)]
            # V_aug: [16 s-tiles][128, 4, 65] bf16; [:, h, 64] = ones
            v_sb = [v_pool.tile([128, HPC, D + 1], BF16, tag=f"v{st}",
                                name=f"v{st}")
                    for st in range(ST)]
            for st in range(ST):
                nc.vector.memset(v_sb[st][:, :, D:D + 1], 1.0)

            def emit_px_dma(nb):
                px_c = [px_pool.tile([128, 512], BF16, tag="px",
                                     name=f"px{kt}_{nb}") for kt in range(KT)]
                for kt in range(KT):
                    nc.sync.dma_start(
                        out=px_c[kt][:],
                        in_=pT[kt * 128:(kt + 1) * 128,
                               nb * 512:(nb + 1) * 512])
                return px_c

            def emit_qx_dma(nb):
                qx_c = [qx_pool.tile([128, 512], BF16, tag="qx",
                                     name=f"qx{kt}_{nb}") for kt in range(KT)]
                for kt in range(KT):
                    nc.sync.dma_start(
                        out=qx_c[kt][:],
                        in_=qT[kt * 128:(kt + 1) * 128,
                               nb * 512:(nb + 1) * 512])
                return qx_c

            def emit_khT(nb, px_c, evict_act=False):
                for mt in range(2):
                    ps = ps_pp.tile([128, 512], F32, tag="pp")
                    for kt in range(KT):
                        nc.tensor.matmul(
                            ps[:],
                            wk_sb[:, kt, mt * 128:(mt + 1) * 128],
                            px_c[kt][:],
                            start=(kt == 0), stop=(kt == KT - 1))
                    dst = khT[mt][:, nb * 512:(nb + 1) * 512]
                    if evict_act:
                        nc.scalar.copy(dst, ps[:])
                    else:
                        nc.vector.tensor_copy(dst, ps[:])

            def emit_qhT(nb, qx_c):
                for mt in range(2):
                    ps = ps_pp.tile([128, 512], F32, tag="pp")
                    for kt in range(KT):
                        nc.tensor.matmul(
                            ps[:],
                            wq_sb[:, kt, mt * 128:(mt + 1) * 128],
                            qx_c[kt][:],
                            start=(kt == 0), stop=(kt == KT - 1))
                    nc.vector.tensor_copy(
                        qhT[mt][:, nb * 512:(nb + 1) * 512], ps[:])

            def emit_V(nb, px_c, evict_act=False):
                for j in range(4):
                    st = nb * 4 + j
                    ps = ps_pp.tile([128, DC], F32, tag="pp")
                    for kt in range(KT):
                        nc.tensor.matmul(
                            ps[:], px_c[kt][:, j * 128:(j + 1) * 128],
                            wv_sb[:, kt, :],
                            start=(kt == 0), stop=(kt == KT - 1))
                    dst = v_sb[st][:, :, 0:D]
                    if evict_act:
                        nc.scalar.copy(dst, ps[:])
                    else:
                        nc.vector.tensor_copy(dst, ps[:])

            def emit_scores(h, ch, st):
                mt, po = h // 2, (h % 2) * 64
                kx_q = qhT[mt][po:po + 64, :]
                kx_k = khT[mt][po:po + 64, :]
                t0 = ch * 1024
                sc = ps_sc.tile([128, 1024], F32, tag="sc")
                for j in range(2):
                    nc.tensor.matmul(
                        sc[:, j * 512:(j + 1) * 512],
                        kx_k[:, st * 128:(st + 1) * 128],
                        kx_q[:, t0 + j * 512:t0 + (j + 1) * 512],
                        start=True, stop=True)
                ptile = pt_pool.tile([128, 1024], BF16, tag="pt")
                nc.scalar.activation(ptile[:], sc[:], EXP, scale=float(SCALE))
                return ptile

            def emit_av(h, st, ptile, o_ps):
                for j in range(2):
                    nc.tensor.matmul(
                        o_ps[j][0:D + 1, :],
                        v_sb[st][:, h, :],
                        ptile[:, j * 512:(j + 1) * 512],
                        start=(st == 0), stop=(st == ST - 1))

            def emit_attn(h, ch, st_lo, st_hi, o_ps):
                for st in range(st_lo, st_hi):
                    ptile = emit_scores(h, ch, st)
                    emit_av(h, st, ptile, o_ps)

            def emit_norm(h, ch, o_ps):
                # per-j chain so each o_ps bank frees as early as possible
                mt, po = h // 2, (h % 2) * 64
                t0 = ch * 1024
                for j in range(2):
                    r_sb = norm_pool.tile([1, 512], F32, tag="r",
                                          name=f"r{h}_{ch}_{j}")
                    nc.vector.reciprocal(r_sb[:], o_ps[j][D:D + 1, :])
                    R_sb = norm_pool.tile([64, 512], F32, tag="R",
                                          name=f"R{h}_{ch}_{j}")
                    nc.gpsimd.partition_broadcast(R_sb[:], r_sb[:])
                    nc.vector.tensor_mul(
                        yT[mt][po:po + 64, t0 + j * 512:t0 + (j + 1) * 512],
                        o_ps[j][0:D, :],
                        R_sb[:])

            def emit_attn_full(h, ch):
                o_ps = [ps_pp.tile([128, 512], F32, tag="pp",
                                  name=f"o{h}_{ch}_{j}") for j in range(2)]
                emit_attn(h, ch, 0, ST, o_ps)
                emit_norm(h, ch, o_ps)

            def emit_outproj(mt2_lo, mt2_hi, act_evict=False):
                for mt2 in range(mt2_lo, mt2_hi):
                    ost = o_pool.tile([128, C], F32, tag="ost",
                                      name=f"ost{mt2}")
                    for nb2 in range(2):
                        if act_evict and mt2 % 2 == 0:
                            ps = ps_sc.tile([128, 512], F32, tag="sc")
                        else:
                            ps = ps_pp.tile([128, 512], F32, tag="pp")
                        for kt2 in range(2):
                            nc.tensor.matmul(
                                ps[:],
                                yT[kt2][:, mt2 * 128:(mt2 + 1) * 128],
                                wo_sb[:, kt2, nb2 * 512:(nb2 + 1) * 512],
                                start=(kt2 == 0), stop=(kt2 == 1))
                        dst = ost[:, nb2 * 512:(nb2 + 1) * 512]
                        if act_evict and mt2 % 2 == 0:
                            nc.scalar.copy(dst, ps[:])
                        else:
                            nc.vector.tensor_copy(dst, ps[:])
                        if act_evict:
                            nc.sync.dma_start(
                                out=out[mt2 * 128:(mt2 + 1) * 128,
                                        nb2 * 512:(nb2 + 1) * 512],
                                in_=dst)
                    if not act_evict:
                        nc.sync.dma_start(
                            out=out[mt2 * 128:(mt2 + 1) * 128, :], in_=ost[:])

            # ---- interleaved emission: overlap DMA / proj / attention /
            # outproj. DMA FIFO order tracks the critical chain to the
            # first exp: wk -> px0 -> qx0/qx1 -> khT0 -> qhT0/1 -> scores.
            warm = norm_pool.tile([1, 8], F32, tag="warm")
            nc.vector.memset(warm[:], 0.0)
            warm2 = norm_pool.tile([1, 8], F32, tag="warm2")
            nc.scalar.activation(warm2[:], warm[:], EXP)  # preload exp table

            emit_w_dma(wk, wk_sb)
            emit_w_dma(wq, wq_sb)
            emit_w_dma(wv, wv_sb)
            nc.sync.dma_start(
                out=wo_sb[:],
                in_=wo.bitcast(F32R).rearrange("(k p) n -> p k n", p=128))
            px0 = emit_px_dma(0)
            qx0 = emit_qx_dma(0)
            qx1 = emit_qx_dma(1)
            px1 = emit_px_dma(1)
            emit_khT(0, px0)
            emit_qhT(0, qx0)
            emit_qhT(1, qx1)
            o_ps00 = [ps_pp.tile([128, 512], F32, tag="pp", name=f"o0_0_{j}")
                      for j in range(2)]
            pt_h = {st: emit_scores(0, 0, st) for st in range(4)}
            emit_khT(1, px1)
            for st in (4, 5, 6, 7):
                pt_h[st] = emit_scores(0, 0, st)
            emit_V(0, px0)
            for st in range(4):
                emit_av(0, st, pt_h.pop(st), o_ps00)
            px2 = emit_px_dma(2)
            emit_khT(2, px2)
            pt_h[8] = emit_scores(0, 0, 8)
            pt_h[9] = emit_scores(0, 0, 9)
            emit_V(1, px1)
            for st in (4, 5):
                emit_av(0, st, pt_h.pop(st), o_ps00)
            for st in (6, 7):
                emit_av(0, st, pt_h.pop(st), o_ps00)
            px3 = emit_px_dma(3)
            emit_khT(3, px3)
            emit_V(2, px2)
            pt_h[10] = emit_scores(0, 0, 10)
            pt_h[11] = emit_scores(0, 0, 11)
            for st in (8, 9):
                emit_av(0, st, pt_h.pop(st), o_ps00)
            emit_V(3, px3)
            pt_h[12] = emit_scores(0, 0, 12)
            pt_h[13] = emit_scores(0, 0, 13)
            for st in (10, 11):
                emit_av(0, st, pt_h.pop(st), o_ps00)
            pt_h[14] = emit_scores(0, 0, 14)
            pt_h[15] = emit_scores(0, 0, 15)
            for st in (12, 13, 14, 15):
                emit_av(0, st, pt_h.pop(st), o_ps00)
            emit_norm(0, 0, o_ps00)
            emit_attn_full(1, 0)
            # qhT for the second t-chunk, deferred out of the DMA-bound front
            qx2 = emit_qx_dma(2)
            qx3 = emit_qx_dma(3)
            emit_qhT(2, qx2)
            emit_attn_full(2, 0)
            emit_qhT(3, qx3)
            emit_attn_full(3, 0)
            emit_attn_full(0, 1)
            emit_outproj(0, 4)
            emit_attn_full(1, 1)
            emit_outproj(4, 8)
            emit_attn_full(2, 1)
            emit_attn_full(3, 1)
            emit_outproj(8, 16, act_evict=True)

    nc.compile()
    return nc


def _get_compiled():
    global _COMPILED
    if _COMPILED is None:
        _COMPILED = _build()
    return _COMPILED


def _make_in_maps(inputs):
    import ml_dtypes
    bf16 = ml_dtypes.bfloat16
    q = np.asarray(inputs["q"], dtype=np.float32)
    p = np.asarray(inputs["p"], dtype=np.float32)
    Wq = np.asarray(inputs["Wq"], dtype=np.float32)
    Wk = np.asarray(inputs["Wk"], dtype=np.float32)
    Wv = np.asarray(inputs["Wv"], dtype=np.float32)
    Wo = np.asarray(inputs["Wo"], dtype=np.float32)
    in_maps = []
    qTs = [np.ascontiguousarray(q[b].T.astype(bf16)) for b in range(B)]
    pTs = [np.ascontiguousarray(p[b].T.astype(bf16)) for b in range(B)]
    for core in range(NCORES):
        b, hg = core // HG, core % HG
        ds = hg * DC
        in_maps.append({
            "qT": qTs[b],
            "pT": pTs[b],
            "wq": np.ascontiguousarray(Wq[:, ds:ds + DC].astype(bf16)),
            "wk": np.ascontiguousarray(Wk[:, ds:ds + DC].astype(bf16)),
            "wv": np.ascontiguousarray(Wv[:, ds:ds + DC].astype(bf16)),
            "wo": np.ascontiguousarray(Wo[ds:ds + DC, :]),
        })
    return in_maps


class _Runner:
    """Caches the compiled NEFF + jitted prep/exec/post programs.

    Per call: ship each input byte to exactly one core (sharded), then
    on-fabric allgather + slice per core (prep jit), run the bass NEFF
    (exec jit), partial-sum the 4 head-group outputs per batch on device
    (post jit), and fetch only 2 of 8 output shards.
    """

    def __init__(self):
        import jax
        import jax.numpy as jnp
        from jax import lax
        from jax.sharding import Mesh, PartitionSpec, NamedSharding
        from jax.experimental.shard_map import shard_map
        from concourse import mybir
        from concourse.bass2jax import (_bass_exec_p, install_neuronx_cc_hook,
                                        partition_id_tensor)

        install_neuronx_cc_hook()
        self.jax = jax
        nc = _get_compiled()
        P = PartitionSpec

        partition_name = (nc.partition_id_tensor.name
                          if nc.partition_id_tensor else None)
        in_names, out_names, out_avals = [], [], []
        for alloc in nc.m.functions[0].allocations:
            if not isinstance(alloc, mybir.MemoryLocationSet):
                continue
            name = alloc.memorylocations[0].name
            if alloc.kind == "ExternalInput":
                if name != partition_name:
                    in_names.append(name)
            elif alloc.kind == "ExternalOutput":
                out_names.append(name)
                out_avals.append(jax.core.ShapedArray(
                    tuple(alloc.tensor_shape), mybir.dt.np(alloc.dtype)))
        all_names = list(in_names) + list(out_names)
        if partition_name is not None:
            all_names.append(partition_name)
        n_params = len(in_names)
        prep_order = ["qT", "pT", "wq", "wk", "wv", "wo"]
        self.perm = [prep_order.index(nm) for nm in in_names]

        devices = jax.devices()[:NCORES]
        mesh = Mesh(__import__("numpy").asarray(devices), ("core",))
        self.mesh = mesh
        self.shard = NamedSharding(mesh, P("core"))

        def prep_body(qT8, pT8, w38, wo8):
            core = lax.axis_index("core")
            b = core // HG
            hg = core % HG
            qT_full = lax.all_gather(qT8, "core", axis=0, tiled=True)
            pT_full = lax.all_gather(pT8, "core", axis=0, tiled=True)
            qT_b = lax.dynamic_index_in_dim(
                qT_full.reshape(B, C, T), b, keepdims=False)
            pT_b = lax.dynamic_index_in_dim(
                pT_full.reshape(B, C, T), b, keepdims=False)
            w3 = lax.all_gather(w38, "core", axis=0, tiled=True)  # [3C, C]
            ds = hg * DC
            wq_s = lax.dynamic_slice(w3, (0, ds), (C, DC))
            wk_s = lax.dynamic_slice(w3, (C, ds), (C, DC))
            wv_s = lax.dynamic_slice(w3, (2 * C, ds), (C, DC))
            wo_full = lax.all_gather(wo8, "core", axis=0, tiled=True)
            wo_s = lax.dynamic_slice(wo_full, (ds, 0), (DC, C))
            zeros = jnp.zeros((T, C), jnp.float32)
            return qT_b, pT_b, wq_s, wk_s, wv_s, wo_s, zeros

        self.prep = jax.jit(shard_map(
            prep_body, mesh=mesh,
            in_specs=(P("core"),) * 4,
            out_specs=(P("core"),) * 7, check_rep=False))

        def bass_body(*args):
            operands = list(args)
            if partition_name is not None:
                operands.append(partition_id_tensor())
            outs = _bass_exec_p.bind(
                *operands, out_avals=tuple(out_avals),
                in_names=tuple(all_names), out_names=tuple(out_names),
                lowering_input_output_aliases=(),
                sim_require_finite=True, sim_require_nnan=True, nc=nc)
            return tuple(outs)

        self.exec = jax.jit(
            shard_map(bass_body, mesh=mesh,
                      in_specs=(P("core"),) * (n_params + 1),
                      out_specs=(P("core"),) * len(out_names),
                      check_rep=False),
            donate_argnums=(n_params,), keep_unused=True)

        groups = [[b * HG + g for g in range(HG)] for b in range(B)]

        def post_body(o):
            return lax.psum(o, "core", axis_index_groups=groups)

        self.post = jax.jit(shard_map(
            post_body, mesh=mesh, in_specs=(P("core"),),
            out_specs=P("core"), check_rep=False))

    def stage(self, inputs):
        import ml_dtypes
        jax = self.jax
        bf16 = ml_dtypes.bfloat16
        q = np.asarray(inputs["q"], dtype=np.float32)
        p = np.asarray(inputs["p"], dtype=np.float32)
        qT8 = np.concatenate(
            [np.ascontiguousarray(q[b].T.astype(bf16)) for b in range(B)],
            axis=0).reshape(NCORES, B * C // NCORES, T)
        pT8 = np.concatenate(
            [np.ascontiguousarray(p[b].T.astype(bf16)) for b in range(B)],
            axis=0).reshape(NCORES, B * C // NCORES, T)
        w38 = np.concatenate(
            [np.asarray(inputs[k], dtype=np.float32).astype(bf16)
             for k in ("Wq", "Wk", "Wv")],
            axis=0).reshape(NCORES, 3 * C // NCORES, C)
        wo8 = np.asarray(inputs["Wo"], dtype=np.float32).reshape(
            NCORES, C // NCORES, C)
        return [jax.device_put(a.reshape(-1, *a.shape[2:]), self.shard)
                for a in (qT8, pT8, w38, wo8)]

    def __call__(self, inputs):
        jax = self.jax
        dev_in = self.stage(inputs)
        prep_out = self.prep(*dev_in)
        ordered = [prep_out[i] for i in self.perm] + [prep_out[6]]
        (bass_out,) = self.exec(*ordered)
        summed = self.post(bass_out)
        out = np.zeros((B, T, C), dtype=np.float32)
        shards = {s.index[0].start or 0: s.data
                  for s in summed.addressable_shards}
        for b in range(B):
            out[b] = np.asarray(shards[b * HG * T])
        return out


_RUNNER = None


def kernel(q, p, Wq, Wk, Wv, Wo):
    global _RUNNER
    inputs = dict(q=q, p=p, Wq=Wq, Wk=Wk, Wv=Wv, Wo=Wo)
    try:
        if _RUNNER is None:
            _RUNNER = _Runner()
        return _RUNNER(inputs)
    except Exception:
        import traceback
        traceback.print_exc()
        return _kernel_fallback(inputs)


def _kernel_fallback(inputs):
    from concourse.bass_utils import run_bass_kernel_spmd

    nc = _get_compiled()
    in_maps = _make_in_maps(inputs)
    res = run_bass_kernel_spmd(nc, in_maps, list(range(NCORES)))
    out = np.zeros((B, T, C), dtype=np.float32)
    for core in range(NCORES):
        out[core // HG] += res.results[core]["out"]
    return out



# revision 3
# speedup vs baseline: 205.3662x; 205.3662x over previous
"""Trainium2 Bass kernel for CrossAttention (B=2, T=S=2048, C=1024, H=16, D=64).

Sharding: 8 cores = 2 batches x 4 head-groups (tensor-parallel over heads,
4 heads per core). Each core computes its heads' attention plus the partial
output projection (Wo row slice); the host sums the 4 partials per batch.

Per-core layout trick: q/p are transposed on the host (qT/pT: [C, T]) so the
contraction dim always lands on SBUF partitions -- no on-chip transposes.

Per-core dataflow (Dc = 256 = 4 heads x 64):
  qhT[Dc,T] = Wq_s.T @ qT     (bf16 in, fp32 accum; stored f32r)
  khT[Dc,S] = Wk_s.T @ pT     (same)
  V[S,Dc]   = pT.T @ Wv_s     (same), stored bf16 with a ones column per
              head (V_aug[s, h, 64] = 1) so the AV matmul also produces the
              softmax row sums (flash-attention style).
  per head h, t-chunk (1024), s-tile (128):
      S^T[s,t] = khT_h.T @ qhT_h          (fp32r, K=64, into PSUM)
      P^T      = exp(0.125 * S^T)         (ACT, PSUM->SBUF, bf16)
      O_aug   += V_aug_h.T @ P^T          (bf16, PSUM accumulation over s)
  normalize: r = 1/rowsum (DVE), broadcast over 64 partitions (gpsimd),
             yT = O * R (DVE, written as f32r)
  out[T, C] = yT.T @ Wo_s  (fp32r) -> DMA; host sums over head groups.
"""

import numpy as np

B, T, S, C, H = 2, 2048, 2048, 1024, 16
D = C // H            # 64
NCORES = 8
HG = 4                # head groups (cores per batch)
HPC = H // HG         # heads per core = 4
DC = HPC * D          # 256 per-core head dims
KT = C // 128         # 8 contraction tiles for projections
NB = T // 512         # 4 free-dim blocks of 512
ST = S // 128         # 16 s-tiles
SCALE = 1.0 / np.sqrt(D)

_COMPILED = None


def _build():
    import concourse.bacc as bacc
    import concourse.tile as tile
    from concourse import mybir

    F32 = mybir.dt.float32
    F32R = mybir.dt.float32r
    BF16 = mybir.dt.bfloat16
    EXP = mybir.ActivationFunctionType.Exp

    nc = bacc.Bacc("TRN2", target_bir_lowering=False, debug=False,
                   num_devices=NCORES)
    qT = nc.dram_tensor("qT", [C, T], BF16, kind="ExternalInput").ap()
    pT = nc.dram_tensor("pT", [C, T], BF16, kind="ExternalInput").ap()
    wq = nc.dram_tensor("wq", [C, DC], BF16, kind="ExternalInput").ap()
    wk = nc.dram_tensor("wk", [C, DC], BF16, kind="ExternalInput").ap()
    wv = nc.dram_tensor("wv", [C, DC], BF16, kind="ExternalInput").ap()
    wo = nc.dram_tensor("wo", [DC, C], F32, kind="ExternalInput").ap()
    out = nc.dram_tensor("out", [T, C], F32, kind="ExternalOutput").ap()

    with tile.TileContext(nc) as tc:
        with (
            tc.tile_pool(name="qx", bufs=32) as qx_pool,         # qT chunks
            tc.tile_pool(name="px", bufs=32) as px_pool,         # pT chunks
            tc.tile_pool(name="w", bufs=1) as w_pool,            # weights
            tc.tile_pool(name="proj", bufs=1) as proj_pool,      # qhT/khT/yT
            tc.tile_pool(name="vaug", bufs=1) as v_pool,
            tc.tile_pool(name="pt", bufs=8) as pt_pool,          # P^T tiles
            tc.tile_pool(name="norm", bufs=2) as norm_pool,
            tc.tile_pool(name="ostage", bufs=3) as o_pool,       # out staging
            tc.tile_pool(name="ps_pp", bufs=4, space="PSUM") as ps_pp,   # 4 banks
            tc.tile_pool(name="ps_sc", bufs=2, space="PSUM") as ps_sc,   # 4 banks
        ):
            # ---- load weights (HWDGE; f32r view of the f32 bits) ----
            wq_sb = w_pool.tile([128, KT, DC], BF16, tag="wq")
            wk_sb = w_pool.tile([128, KT, DC], BF16, tag="wk")
            wv_sb = w_pool.tile([128, KT, DC], BF16, tag="wv")
            wo_sb = w_pool.tile([128, 2, C], F32R, tag="wo")

            def emit_w_dma(w_dram, w_sb):
                nc.sync.dma_start(
                    out=w_sb[:],
                    in_=w_dram.rearrange("(k p) n -> p k n", p=128))

            # qhT/khT/yT: [2 x 128 d, T]; heads 2m,2m+1 in tile m at part 0/64
            qhT = [proj_pool.tile([128, T], F32R, tag=f"qhT{m}", name=f"qhT{m}")
                   for m in range(2)]
            khT = [proj_pool.tile([128, T], F32R, tag=f"khT{m}", name=f"khT{m}")
                   for m in range(2)]
            yT = [proj_pool.tile([128, T], F32R, tag=f"yT{m}", name=f"yT{m}")
                  for m in range(2)]
            # V_aug: [16 s-tiles][128, 4, 65] bf16; [:, h, 64] = ones
            v_sb = [v_pool.tile([128, HPC, D + 1], BF16, tag=f"v{st}",
                                name=f"v{st}")
                    for st in range(ST)]
            for st in range(ST):
                nc.vector.memset(v_sb[st][:, :, D:D + 1], 1.0)

            def emit_px_dma(nb):
                px_c = [px_pool.tile([128, 512], BF16, tag="px",
                                     name=f"px{kt}_{nb}") for kt in range(KT)]
                for kt in range(KT):
                    nc.sync.dma_start(
                        out=px_c[kt][:],
                        in_=pT[kt * 128:(kt + 1) * 128,
                               nb * 512:(nb + 1) * 512])
                return px_c

            def emit_qx_dma(nb):
                qx_c = [qx_pool.tile([128, 512], BF16, tag="qx",
                                     name=f"qx{kt}_{nb}") for kt in range(KT)]
                for kt in range(KT):
                    nc.sync.dma_start(
                        out=qx_c[kt][:],
                        in_=qT[kt * 128:(kt + 1) * 128,
                               nb * 512:(nb + 1) * 512])
                return qx_c

            def emit_khT(nb, px_c, evict_act=False):
                for mt in range(2):
                    ps = ps_pp.tile([128, 512], F32, tag="pp")
                    for kt in range(KT):
                        nc.tensor.matmul(
                            ps[:],
                            wk_sb[:, kt, mt * 128:(mt + 1) * 128],
                            px_c[kt][:],
                            start=(kt == 0), stop=(kt == KT - 1))
                    dst = khT[mt][:, nb * 512:(nb + 1) * 512]
                    if evict_act:
                        nc.scalar.copy(dst, ps[:])
                    else:
                        nc.vector.tensor_copy(dst, ps[:])

            def emit_qhT(nb, qx_c):
                for mt in range(2):
                    ps = ps_pp.tile([128, 512], F32, tag="pp")
                    for kt in range(KT):
                        nc.tensor.matmul(
                            ps[:],
                            wq_sb[:, kt, mt * 128:(mt + 1) * 128],
                            qx_c[kt][:],
                            start=(kt == 0), stop=(kt == KT - 1))
                    nc.vector.tensor_copy(
                        qhT[mt][:, nb * 512:(nb + 1) * 512], ps[:])

            def emit_V(nb, px_c, evict_act=False):
                for j in range(4):
                    st = nb * 4 + j
                    ps = ps_pp.tile([128, DC], F32, tag="pp")
                    for kt in range(KT):
                        nc.tensor.matmul(
                            ps[:], px_c[kt][:, j * 128:(j + 1) * 128],
                            wv_sb[:, kt, :],
                            start=(kt == 0), stop=(kt == KT - 1))
                    dst = v_sb[st][:, :, 0:D]
                    if evict_act:
                        nc.scalar.copy(dst, ps[:])
                    else:
                        nc.vector.tensor_copy(dst, ps[:])

            def emit_scores(h, ch, st):
                mt, po = h // 2, (h % 2) * 64
                kx_q = qhT[mt][po:po + 64, :]
                kx_k = khT[mt][po:po + 64, :]
                t0 = ch * 1024
                sc = ps_sc.tile([128, 1024], F32, tag="sc")
                for j in range(2):
                    nc.tensor.matmul(
                        sc[:, j * 512:(j + 1) * 512],
                        kx_k[:, st * 128:(st + 1) * 128],
                        kx_q[:, t0 + j * 512:t0 + (j + 1) * 512],
                        start=True, stop=True)
                ptile = pt_pool.tile([128, 1024], BF16, tag="pt")
                nc.scalar.activation(ptile[:], sc[:], EXP, scale=float(SCALE))
                return ptile

            def emit_av(h, st, ptile, o_ps):
                for j in range(2):
                    nc.tensor.matmul(
                        o_ps[j][0:D + 1, :],
                        v_sb[st][:, h, :],
                        ptile[:, j * 512:(j + 1) * 512],
                        start=(st == 0), stop=(st == ST - 1))

            def emit_attn(h, ch, st_lo, st_hi, o_ps):
                for st in range(st_lo, st_hi):
                    ptile = emit_scores(h, ch, st)
                    emit_av(h, st, ptile, o_ps)

            def emit_norm(h, ch, o_ps):
                # per-j chain so each o_ps bank frees as early as possible
                mt, po = h // 2, (h % 2) * 64
                t0 = ch * 1024
                for j in range(2):
                    r_sb = norm_pool.tile([1, 512], F32, tag="r",
                                          name=f"r{h}_{ch}_{j}")
                    nc.vector.reciprocal(r_sb[:], o_ps[j][D:D + 1, :])
                    R_sb = norm_pool.tile([64, 512], F32, tag="R",
                                          name=f"R{h}_{ch}_{j}")
                    nc.gpsimd.partition_broadcast(R_sb[:], r_sb[:])
                    nc.vector.tensor_mul(
                        yT[mt][po:po + 64, t0 + j * 512:t0 + (j + 1) * 512],
                        o_ps[j][0:D, :],
                        R_sb[:])

            def emit_attn_full(h, ch):
                o_ps = [ps_pp.tile([128, 512], F32, tag="pp",
                                  name=f"o{h}_{ch}_{j}") for j in range(2)]
                emit_attn(h, ch, 0, ST, o_ps)
                emit_norm(h, ch, o_ps)

            def emit_outproj(mt2_lo, mt2_hi, act_evict=False):
                for mt2 in range(mt2_lo, mt2_hi):
                    ost = o_pool.tile([128, C], F32, tag="ost",
                                      name=f"ost{mt2}")
                    for nb2 in range(2):
                        if act_evict and mt2 % 2 == 0:
                            ps = ps_sc.tile([128, 512], F32, tag="sc")
                        else:
                            ps = ps_pp.tile([128, 512], F32, tag="pp")
                        for kt2 in range(2):
                            nc.tensor.matmul(
                                ps[:],
                                yT[kt2][:, mt2 * 128:(mt2 + 1) * 128],
                                wo_sb[:, kt2, nb2 * 512:(nb2 + 1) * 512],
                                start=(kt2 == 0), stop=(kt2 == 1))
                        dst = ost[:, nb2 * 512:(nb2 + 1) * 512]
                        if act_evict and mt2 % 2 == 0:
                            nc.scalar.copy(dst, ps[:])
                        else:
                            nc.vector.tensor_copy(dst, ps[:])
                        if act_evict:
                            nc.sync.dma_start(
                                out=out[mt2 * 128:(mt2 + 1) * 128,
                                        nb2 * 512:(nb2 + 1) * 512],
                                in_=dst)
                    if not act_evict:
                        nc.sync.dma_start(
                            out=out[mt2 * 128:(mt2 + 1) * 128, :], in_=ost[:])

            # ---- interleaved emission: overlap DMA / proj / attention /
            # outproj. DMA FIFO order tracks the critical chain to the
            # first exp: wk -> px0 -> qx0/qx1 -> khT0 -> qhT0/1 -> scores.
            warm = norm_pool.tile([1, 8], F32, tag="warm")
            nc.vector.memset(warm[:], 0.0)
            warm2 = norm_pool.tile([1, 8], F32, tag="warm2")
            nc.scalar.activation(warm2[:], warm[:], EXP)  # preload exp table

            emit_w_dma(wk, wk_sb)
            emit_w_dma(wq, wq_sb)
            emit_w_dma(wv, wv_sb)
            nc.sync.dma_start(
                out=wo_sb[:],
                in_=wo.bitcast(F32R).rearrange("(k p) n -> p k n", p=128))
            px0 = emit_px_dma(0)
            qx0 = emit_qx_dma(0)
            qx1 = emit_qx_dma(1)
            px1 = emit_px_dma(1)
            emit_khT(0, px0)
            emit_qhT(0, qx0)
            emit_qhT(1, qx1)
            o_ps00 = [ps_pp.tile([128, 512], F32, tag="pp", name=f"o0_0_{j}")
                      for j in range(2)]
            pt_h = {st: emit_scores(0, 0, st) for st in range(4)}
            emit_khT(1, px1)
            for st in (4, 5, 6, 7):
                pt_h[st] = emit_scores(0, 0, st)
            emit_V(0, px0)
            for st in range(4):
                emit_av(0, st, pt_h.pop(st), o_ps00)
            px2 = emit_px_dma(2)
            emit_khT(2, px2)
            pt_h[8] = emit_scores(0, 0, 8)
            pt_h[9] = emit_scores(0, 0, 9)
            emit_V(1, px1)
            for st in (4, 5):
                emit_av(0, st, pt_h.pop(st), o_ps00)
            for st in (6, 7):
                emit_av(0, st, pt_h.pop(st), o_ps00)
            px3 = emit_px_dma(3)
            emit_khT(3, px3)
            emit_V(2, px2)
            pt_h[10] = emit_scores(0, 0, 10)
            pt_h[11] = emit_scores(0, 0, 11)
            for st in (8, 9):
                emit_av(0, st, pt_h.pop(st), o_ps00)
            emit_V(3, px3)
            pt_h[12] = emit_scores(0, 0, 12)
            pt_h[13] = emit_scores(0, 0, 13)
            for st in (10, 11):
                emit_av(0, st, pt_h.pop(st), o_ps00)
            pt_h[14] = emit_scores(0, 0, 14)
            pt_h[15] = emit_scores(0, 0, 15)
            for st in (12, 13, 14, 15):
                emit_av(0, st, pt_h.pop(st), o_ps00)
            emit_norm(0, 0, o_ps00)
            emit_attn_full(1, 0)
            # qhT for the second t-chunk, deferred out of the DMA-bound front
            qx2 = emit_qx_dma(2)
            qx3 = emit_qx_dma(3)
            emit_qhT(2, qx2)
            emit_attn_full(2, 0)
            emit_qhT(3, qx3)
            emit_attn_full(3, 0)
            emit_attn_full(0, 1)
            emit_outproj(0, 4)
            emit_attn_full(1, 1)
            emit_outproj(4, 8)
            emit_attn_full(2, 1)
            emit_attn_full(3, 1)
            emit_outproj(8, 16, act_evict=True)

    nc.compile()
    return nc


def _get_compiled():
    global _COMPILED
    if _COMPILED is None:
        _COMPILED = _build()
    return _COMPILED


def _make_in_maps(inputs):
    import ml_dtypes
    bf16 = ml_dtypes.bfloat16
    q = np.asarray(inputs["q"], dtype=np.float32)
    p = np.asarray(inputs["p"], dtype=np.float32)
    Wq = np.asarray(inputs["Wq"], dtype=np.float32)
    Wk = np.asarray(inputs["Wk"], dtype=np.float32)
    Wv = np.asarray(inputs["Wv"], dtype=np.float32)
    Wo = np.asarray(inputs["Wo"], dtype=np.float32)
    in_maps = []
    qTs = [np.ascontiguousarray(q[b].T.astype(bf16)) for b in range(B)]
    pTs = [np.ascontiguousarray(p[b].T.astype(bf16)) for b in range(B)]
    for core in range(NCORES):
        b, hg = core // HG, core % HG
        ds = hg * DC
        in_maps.append({
            "qT": qTs[b],
            "pT": pTs[b],
            "wq": np.ascontiguousarray(Wq[:, ds:ds + DC].astype(bf16)),
            "wk": np.ascontiguousarray(Wk[:, ds:ds + DC].astype(bf16)),
            "wv": np.ascontiguousarray(Wv[:, ds:ds + DC].astype(bf16)),
            "wo": np.ascontiguousarray(Wo[ds:ds + DC, :]),
        })
    return in_maps


class _Runner:
    """Caches the compiled NEFF + jitted prep/exec/post programs.

    Per call: ship each input byte to exactly one core (sharded), then
    on-fabric allgather + slice per core (prep jit), run the bass NEFF
    (exec jit), partial-sum the 4 head-group outputs per batch on device
    (post jit), and fetch only 2 of 8 output shards.
    """

    def __init__(self):
        import jax
        import jax.numpy as jnp
        from jax import lax
        from jax.sharding import Mesh, PartitionSpec, NamedSharding
        from jax.experimental.shard_map import shard_map
        from concourse import mybir
        from concourse.bass2jax import (_bass_exec_p, fast_dispatch_compile,
                                        install_neuronx_cc_hook,
                                        partition_id_tensor)

        install_neuronx_cc_hook()
        self.jax = jax
        nc = _get_compiled()
        P = PartitionSpec

        partition_name = (nc.partition_id_tensor.name
                          if nc.partition_id_tensor else None)
        in_names, out_names, out_avals = [], [], []
        for alloc in nc.m.functions[0].allocations:
            if not isinstance(alloc, mybir.MemoryLocationSet):
                continue
            name = alloc.memorylocations[0].name
            if alloc.kind == "ExternalInput":
                if name != partition_name:
                    in_names.append(name)
            elif alloc.kind == "ExternalOutput":
                out_names.append(name)
                out_avals.append(jax.core.ShapedArray(
                    tuple(alloc.tensor_shape), mybir.dt.np(alloc.dtype)))
        all_names = list(in_names) + list(out_names)
        if partition_name is not None:
            all_names.append(partition_name)
        n_params = len(in_names)
        prep_order = ["qT", "pT", "wq", "wk", "wv", "wo"]
        self.perm = [prep_order.index(nm) for nm in in_names]

        devices = jax.devices()[:NCORES]
        mesh = Mesh(__import__("numpy").asarray(devices), ("core",))
        self.mesh = mesh
        self.shard = NamedSharding(mesh, P("core"))

        def prep_body(qT8, pT8, w38, wo8):
            core = lax.axis_index("core")
            b = core // HG
            hg = core % HG
            qT_full = lax.all_gather(qT8, "core", axis=0, tiled=True)
            pT_full = lax.all_gather(pT8, "core", axis=0, tiled=True)
            qT_b = lax.dynamic_index_in_dim(
                qT_full.reshape(B, C, T), b, keepdims=False)
            pT_b = lax.dynamic_index_in_dim(
                pT_full.reshape(B, C, T), b, keepdims=False)
            w3 = lax.all_gather(w38, "core", axis=0, tiled=True)  # [3C, C]
            ds = hg * DC
            wq_s = lax.dynamic_slice(w3, (0, ds), (C, DC))
            wk_s = lax.dynamic_slice(w3, (C, ds), (C, DC))
            wv_s = lax.dynamic_slice(w3, (2 * C, ds), (C, DC))
            wo_full = lax.all_gather(wo8, "core", axis=0, tiled=True)
            wo_s = lax.dynamic_slice(wo_full, (ds, 0), (DC, C))
            zeros = jnp.zeros((T, C), jnp.float32)
            return qT_b, pT_b, wq_s, wk_s, wv_s, wo_s, zeros

        self.prep = jax.jit(shard_map(
            prep_body, mesh=mesh,
            in_specs=(P("core"),) * 4,
            out_specs=(P("core"),) * 7, check_rep=False))

        def bass_body(*args):
            operands = list(args)
            if partition_name is not None:
                operands.append(partition_id_tensor())
            outs = _bass_exec_p.bind(
                *operands, out_avals=tuple(out_avals),
                in_names=tuple(all_names), out_names=tuple(out_names),
                lowering_input_output_aliases=(),
                sim_require_finite=True, sim_require_nnan=True, nc=nc)
            return tuple(outs)

        # Fast-dispatch (C++ dispatch path, no effect token) + no donation:
        # the bass kernel writes every byte of `out`, so the donated-zeros
        # aliasing is unnecessary; without it the same device-resident
        # argument buffers can be re-executed back-to-back, which both the
        # steady-state benchmark in test.py and repeat kernel() calls use.
        # (Output equality with the donated path was verified bit-for-bit.)
        shapes_by_name = {
            "qT": ((C, T), jnp.bfloat16), "pT": ((C, T), jnp.bfloat16),
            "wq": ((C, DC), jnp.bfloat16), "wk": ((C, DC), jnp.bfloat16),
            "wv": ((C, DC), jnp.bfloat16), "wo": ((DC, C), jnp.float32),
            "out": ((T, C), jnp.float32),
        }
        arg_structs = [
            jax.ShapeDtypeStruct(
                (NCORES * shapes_by_name[nm][0][0],) + shapes_by_name[nm][0][1:],
                shapes_by_name[nm][1], sharding=self.shard)
            for nm in list(in_names) + ["out"]
        ]

        def compile_exec():
            f = jax.jit(
                shard_map(bass_body, mesh=mesh,
                          in_specs=(P("core"),) * (n_params + 1),
                          out_specs=(P("core"),) * len(out_names),
                          check_rep=False),
                keep_unused=True)
            return f.lower(*arg_structs).compile()

        self.exec = fast_dispatch_compile(compile_exec)

        groups = [[b * HG + g for g in range(HG)] for b in range(B)]

        def post_body(o):
            return lax.psum(o, "core", axis_index_groups=groups)

        self.post = jax.jit(shard_map(
            post_body, mesh=mesh, in_specs=(P("core"),),
            out_specs=P("core"), check_rep=False))

    def stage(self, inputs):
        import ml_dtypes
        jax = self.jax
        bf16 = ml_dtypes.bfloat16
        q = np.asarray(inputs["q"], dtype=np.float32)
        p = np.asarray(inputs["p"], dtype=np.float32)
        qT8 = np.concatenate(
            [np.ascontiguousarray(q[b].T.astype(bf16)) for b in range(B)],
            axis=0).reshape(NCORES, B * C // NCORES, T)
        pT8 = np.concatenate(
            [np.ascontiguousarray(p[b].T.astype(bf16)) for b in range(B)],
            axis=0).reshape(NCORES, B * C // NCORES, T)
        w38 = np.concatenate(
            [np.asarray(inputs[k], dtype=np.float32).astype(bf16)
             for k in ("Wq", "Wk", "Wv")],
            axis=0).reshape(NCORES, 3 * C // NCORES, C)
        wo8 = np.asarray(inputs["Wo"], dtype=np.float32).reshape(
            NCORES, C // NCORES, C)
        return [jax.device_put(a.reshape(-1, *a.shape[2:]), self.shard)
                for a in (qT8, pT8, w38, wo8)]

    def __call__(self, inputs):
        jax = self.jax
        dev_in = self.stage(inputs)
        prep_out = self.prep(*dev_in)
        ordered = [prep_out[i] for i in self.perm] + [prep_out[6]]
        (bass_out,) = self.exec(*ordered)
        summed = self.post(bass_out)
        out = np.zeros((B, T, C), dtype=np.float32)
        shards = {s.index[0].start or 0: s.data
                  for s in summed.addressable_shards}
        for b in range(B):
            out[b] = np.asarray(shards[b * HG * T])
        return out


_RUNNER = None


def kernel(q, p, Wq, Wk, Wv, Wo):
    global _RUNNER
    inputs = dict(q=q, p=p, Wq=Wq, Wk=Wk, Wv=Wv, Wo=Wo)
    try:
        if _RUNNER is None:
            _RUNNER = _Runner()
        return _RUNNER(inputs)
    except Exception:
        import traceback
        traceback.print_exc()
        return _kernel_fallback(inputs)


def _kernel_fallback(inputs):
    from concourse.bass_utils import run_bass_kernel_spmd

    nc = _get_compiled()
    in_maps = _make_in_maps(inputs)
    res = run_bass_kernel_spmd(nc, in_maps, list(range(NCORES)))
    out = np.zeros((B, T, C), dtype=np.float32)
    for core in range(NCORES):
        out[core // HG] += res.results[core]["out"]
    return out



# revision 15
# speedup vs baseline: 205.8484x; 1.0023x over previous
"""Trainium2 Bass kernel for CrossAttention (B=2, T=S=2048, C=1024, H=16, D=64).

Sharding: 8 cores = 2 batches x 4 head-groups (tensor-parallel over heads,
4 heads per core). Each core computes its heads' attention plus the partial
output projection (Wo row slice); the host sums the 4 partials per batch.

Per-core layout trick: q/p are transposed on the host (qT/pT: [C, T]) so the
contraction dim always lands on SBUF partitions -- no on-chip transposes.

Per-core dataflow (Dc = 256 = 4 heads x 64):
  qhT[Dc,T] = Wq_s.T @ qT     (bf16 in, fp32 accum; stored f32r)
  khT[Dc,S] = Wk_s.T @ pT     (same)
  V[S,Dc]   = pT.T @ Wv_s     (same), stored bf16 with a ones column per
              head (V_aug[s, h, 64] = 1) so the AV matmul also produces the
              softmax row sums (flash-attention style).
  per head h, t-chunk (1024), s-tile (128):
      S^T[s,t] = khT_h.T @ qhT_h          (fp32r, K=64, into PSUM)
      P^T      = exp(0.125 * S^T)         (ACT, PSUM->SBUF, bf16)
      O_aug   += V_aug_h.T @ P^T          (bf16, PSUM accumulation over s)
  normalize: r = 1/rowsum (DVE), broadcast over 64 partitions (gpsimd),
             yT = O * R (DVE, written as f32r)
  out[T, C] = yT.T @ Wo_s  (fp32r) -> DMA; host sums over head groups.
"""

import numpy as np

B, T, S, C, H = 2, 2048, 2048, 1024, 16
D = C // H            # 64
NCORES = 8
HG = 4                # head groups (cores per batch)
HPC = H // HG         # heads per core = 4
DC = HPC * D          # 256 per-core head dims
KT = C // 128         # 8 contraction tiles for projections
NB = T // 512         # 4 free-dim blocks of 512
ST = S // 128         # 16 s-tiles
SCALE = 1.0 / np.sqrt(D)

_COMPILED = None


def _build():
    import concourse.bacc as bacc
    import concourse.tile as tile
    from concourse import mybir

    F32 = mybir.dt.float32
    F32R = mybir.dt.float32r
    BF16 = mybir.dt.bfloat16
    EXP = mybir.ActivationFunctionType.Exp

    nc = bacc.Bacc("TRN2", target_bir_lowering=False, debug=False,
                   num_devices=NCORES)
    qT = nc.dram_tensor("qT", [C, T], BF16, kind="ExternalInput").ap()
    pT = nc.dram_tensor("pT", [C, T], BF16, kind="ExternalInput").ap()
    wq = nc.dram_tensor("wq", [C, DC], BF16, kind="ExternalInput").ap()
    wk = nc.dram_tensor("wk", [C, DC], BF16, kind="ExternalInput").ap()
    wv = nc.dram_tensor("wv", [C, DC], BF16, kind="ExternalInput").ap()
    wo = nc.dram_tensor("wo", [DC, C], BF16, kind="ExternalInput").ap()
    out = nc.dram_tensor("out", [T, C], F32, kind="ExternalOutput").ap()

    with tile.TileContext(nc) as tc:
        with (
            tc.tile_pool(name="qx", bufs=32) as qx_pool,         # qT chunks
            tc.tile_pool(name="px", bufs=32) as px_pool,         # pT chunks
            tc.tile_pool(name="w", bufs=1) as w_pool,            # weights
            tc.tile_pool(name="proj", bufs=1) as proj_pool,      # qhT/khT/yT
            tc.tile_pool(name="vaug", bufs=1) as v_pool,
            tc.tile_pool(name="pt", bufs=8) as pt_pool,          # P^T tiles
            tc.tile_pool(name="norm", bufs=2) as norm_pool,
            tc.tile_pool(name="ostage", bufs=3) as o_pool,       # out staging
            tc.tile_pool(name="ps_pp", bufs=4, space="PSUM") as ps_pp,   # 4 banks
            tc.tile_pool(name="ps_sc", bufs=2, space="PSUM") as ps_sc,   # 4 banks
        ):
            # ---- load weights (HWDGE; f32r view of the f32 bits) ----
            wq_sb = w_pool.tile([128, KT, DC], BF16, tag="wq")
            wk_sb = w_pool.tile([128, KT, DC], BF16, tag="wk")
            wv_sb = w_pool.tile([128, KT, DC], BF16, tag="wv")
            wo_sb = w_pool.tile([128, 2, C], BF16, tag="wo")

            def emit_w_dma(w_dram, w_sb):
                nc.sync.dma_start(
                    out=w_sb[:],
                    in_=w_dram.rearrange("(k p) n -> p k n", p=128))

            # qhT/khT/yT: [2 x 128 d, T]; heads 2m,2m+1 in tile m at part 0/64
            qhT = [proj_pool.tile([128, T], BF16, tag=f"qhT{m}", name=f"qhT{m}")
                   for m in range(2)]
            khT = [proj_pool.tile([128, T], BF16, tag=f"khT{m}", name=f"khT{m}")
                   for m in range(2)]
            yT = [proj_pool.tile([128, T], BF16, tag=f"yT{m}", name=f"yT{m}")
                  for m in range(2)]
            # V_aug: [16 s-tiles][128, 4, 65] bf16; [:, h, 64] = ones
            v_sb = [v_pool.tile([128, HPC, D + 1], BF16, tag=f"v{st}",
                                name=f"v{st}")
                    for st in range(ST)]
            for st in range(ST):
                nc.vector.memset(v_sb[st][:, :, D:D + 1], 1.0)

            def emit_px_dma(nb):
                px_c = [px_pool.tile([128, 512], BF16, tag="px",
                                     name=f"px{kt}_{nb}") for kt in range(KT)]
                for kt in range(KT):
                    nc.sync.dma_start(
                        out=px_c[kt][:],
                        in_=pT[kt * 128:(kt + 1) * 128,
                               nb * 512:(nb + 1) * 512])
                return px_c

            def emit_qx_dma(nb):
                qx_c = [qx_pool.tile([128, 512], BF16, tag="qx",
                                     name=f"qx{kt}_{nb}") for kt in range(KT)]
                for kt in range(KT):
                    nc.sync.dma_start(
                        out=qx_c[kt][:],
                        in_=qT[kt * 128:(kt + 1) * 128,
                               nb * 512:(nb + 1) * 512])
                return qx_c

            def emit_khT(nb, px_c, evict_act=False):
                for mt in range(2):
                    ps = ps_pp.tile([128, 512], F32, tag="pp")
                    for kt in range(KT):
                        nc.tensor.matmul(
                            ps[:],
                            wk_sb[:, kt, mt * 128:(mt + 1) * 128],
                            px_c[kt][:],
                            start=(kt == 0), stop=(kt == KT - 1))
                    dst = khT[mt][:, nb * 512:(nb + 1) * 512]
                    if evict_act:
                        nc.scalar.copy(dst, ps[:])
                    else:
                        nc.vector.tensor_copy(dst, ps[:])

            def emit_qhT(nb, qx_c):
                for mt in range(2):
                    ps = ps_pp.tile([128, 512], F32, tag="pp")
                    for kt in range(KT):
                        nc.tensor.matmul(
                            ps[:],
                            wq_sb[:, kt, mt * 128:(mt + 1) * 128],
                            qx_c[kt][:],
                            start=(kt == 0), stop=(kt == KT - 1))
                    nc.vector.tensor_copy(
                        qhT[mt][:, nb * 512:(nb + 1) * 512], ps[:])

            def emit_V(nb, px_c, evict_act=False):
                for j in range(4):
                    st = nb * 4 + j
                    ps = ps_pp.tile([128, DC], F32, tag="pp")
                    for kt in range(KT):
                        nc.tensor.matmul(
                            ps[:], px_c[kt][:, j * 128:(j + 1) * 128],
                            wv_sb[:, kt, :],
                            start=(kt == 0), stop=(kt == KT - 1))
                    dst = v_sb[st][:, :, 0:D]
                    if evict_act:
                        nc.scalar.copy(dst, ps[:])
                    else:
                        nc.vector.tensor_copy(dst, ps[:])

            def emit_scores(h, ch, st):
                mt, po = h // 2, (h % 2) * 64
                kx_q = qhT[mt][po:po + 64, :]
                kx_k = khT[mt][po:po + 64, :]
                t0 = ch * 1024
                sc = ps_sc.tile([128, 1024], F32, tag="sc")
                for j in range(2):
                    nc.tensor.matmul(
                        sc[:, j * 512:(j + 1) * 512],
                        kx_k[:, st * 128:(st + 1) * 128],
                        kx_q[:, t0 + j * 512:t0 + (j + 1) * 512],
                        start=True, stop=True)
                ptile = pt_pool.tile([128, 1024], BF16, tag="pt")
                nc.scalar.activation(ptile[:], sc[:], EXP, scale=float(SCALE))
                return ptile

            def emit_av(h, st, ptile, o_ps):
                for j in range(2):
                    nc.tensor.matmul(
                        o_ps[j][0:D + 1, :],
                        v_sb[st][:, h, :],
                        ptile[:, j * 512:(j + 1) * 512],
                        start=(st == 0), stop=(st == ST - 1))

            def emit_attn(h, ch, st_lo, st_hi, o_ps):
                for st in range(st_lo, st_hi):
                    ptile = emit_scores(h, ch, st)
                    emit_av(h, st, ptile, o_ps)

            def emit_norm(h, ch, o_ps):
                # per-j chain so each o_ps bank frees as early as possible
                mt, po = h // 2, (h % 2) * 64
                t0 = ch * 1024
                for j in range(2):
                    r_sb = norm_pool.tile([1, 512], F32, tag="r",
                                          name=f"r{h}_{ch}_{j}")
                    nc.vector.reciprocal(r_sb[:], o_ps[j][D:D + 1, :])
                    R_sb = norm_pool.tile([64, 512], F32, tag="R",
                                          name=f"R{h}_{ch}_{j}")
                    nc.gpsimd.partition_broadcast(R_sb[:], r_sb[:])
                    nc.vector.tensor_mul(
                        yT[mt][po:po + 64, t0 + j * 512:t0 + (j + 1) * 512],
                        o_ps[j][0:D, :],
                        R_sb[:])

            def emit_attn_full(h, ch):
                o_ps = [ps_pp.tile([128, 512], F32, tag="pp",
                                  name=f"o{h}_{ch}_{j}") for j in range(2)]
                emit_attn(h, ch, 0, ST, o_ps)
                emit_norm(h, ch, o_ps)

            def emit_outproj(mt2_lo, mt2_hi, act_evict=False):
                for mt2 in range(mt2_lo, mt2_hi):
                    ost = o_pool.tile([128, C], F32, tag="ost",
                                      name=f"ost{mt2}")
                    for nb2 in range(2):
                        if act_evict and mt2 % 2 == 0:
                            ps = ps_sc.tile([128, 512], F32, tag="sc")
                        else:
                            ps = ps_pp.tile([128, 512], F32, tag="pp")
                        for kt2 in range(2):
                            nc.tensor.matmul(
                                ps[:],
                                yT[kt2][:, mt2 * 128:(mt2 + 1) * 128],
                                wo_sb[:, kt2, nb2 * 512:(nb2 + 1) * 512],
                                start=(kt2 == 0), stop=(kt2 == 1))
                        dst = ost[:, nb2 * 512:(nb2 + 1) * 512]
                        if act_evict and mt2 % 2 == 0:
                            nc.scalar.copy(dst, ps[:])
                        else:
                            nc.vector.tensor_copy(dst, ps[:])
                        if act_evict:
                            nc.sync.dma_start(
                                out=out[mt2 * 128:(mt2 + 1) * 128,
                                        nb2 * 512:(nb2 + 1) * 512],
                                in_=dst)
                    if not act_evict:
                        nc.sync.dma_start(
                            out=out[mt2 * 128:(mt2 + 1) * 128, :], in_=ost[:])

            # ---- interleaved emission: overlap DMA / proj / attention /
            # outproj. DMA FIFO order tracks the critical chain to the
            # first exp: wk -> px0 -> qx0/qx1 -> khT0 -> qhT0/1 -> scores.
            warm = norm_pool.tile([1, 8], F32, tag="warm")
            nc.vector.memset(warm[:], 0.0)
            warm2 = norm_pool.tile([1, 8], F32, tag="warm2")
            nc.scalar.activation(warm2[:], warm[:], EXP)  # preload exp table

            emit_w_dma(wk, wk_sb)
            emit_w_dma(wq, wq_sb)
            emit_w_dma(wv, wv_sb)
            nc.sync.dma_start(
                out=wo_sb[:],
                in_=wo.rearrange("(k p) n -> p k n", p=128))
            px0 = emit_px_dma(0)
            qx0 = emit_qx_dma(0)
            qx1 = emit_qx_dma(1)
            px1 = emit_px_dma(1)
            emit_khT(0, px0)
            emit_qhT(0, qx0)
            emit_qhT(1, qx1)
            o_ps00 = [ps_pp.tile([128, 512], F32, tag="pp", name=f"o0_0_{j}")
                      for j in range(2)]
            pt_h = {st: emit_scores(0, 0, st) for st in range(4)}
            emit_khT(1, px1)
            for st in (4, 5, 6, 7):
                pt_h[st] = emit_scores(0, 0, st)
            emit_V(0, px0)
            for st in range(4):
                emit_av(0, st, pt_h.pop(st), o_ps00)
            px2 = emit_px_dma(2)
            emit_khT(2, px2)
            pt_h[8] = emit_scores(0, 0, 8)
            pt_h[9] = emit_scores(0, 0, 9)
            emit_V(1, px1)
            for st in (4, 5):
                emit_av(0, st, pt_h.pop(st), o_ps00)
            for st in (6, 7):
                emit_av(0, st, pt_h.pop(st), o_ps00)
            px3 = emit_px_dma(3)
            emit_khT(3, px3)
            emit_V(2, px2)
            pt_h[10] = emit_scores(0, 0, 10)
            pt_h[11] = emit_scores(0, 0, 11)
            for st in (8, 9):
                emit_av(0, st, pt_h.pop(st), o_ps00)
            emit_V(3, px3)
            pt_h[12] = emit_scores(0, 0, 12)
            pt_h[13] = emit_scores(0, 0, 13)
            for st in (10, 11):
                emit_av(0, st, pt_h.pop(st), o_ps00)
            pt_h[14] = emit_scores(0, 0, 14)
            pt_h[15] = emit_scores(0, 0, 15)
            for st in (12, 13, 14, 15):
                emit_av(0, st, pt_h.pop(st), o_ps00)
            emit_norm(0, 0, o_ps00)
            emit_attn_full(1, 0)
            # qhT for the second t-chunk, deferred out of the DMA-bound front
            qx2 = emit_qx_dma(2)
            qx3 = emit_qx_dma(3)
            emit_qhT(2, qx2)
            emit_attn_full(2, 0)
            emit_qhT(3, qx3)
            emit_attn_full(3, 0)
            emit_attn_full(0, 1)
            emit_outproj(0, 4)
            emit_attn_full(1, 1)
            emit_outproj(4, 8)
            emit_attn_full(2, 1)
            emit_attn_full(3, 1)
            emit_outproj(8, 16, act_evict=True)

    nc.compile()
    return nc


def _get_compiled():
    global _COMPILED
    if _COMPILED is None:
        _COMPILED = _build()
    return _COMPILED


def _make_in_maps(inputs):
    import ml_dtypes
    bf16 = ml_dtypes.bfloat16
    q = np.asarray(inputs["q"], dtype=np.float32)
    p = np.asarray(inputs["p"], dtype=np.float32)
    Wq = np.asarray(inputs["Wq"], dtype=np.float32)
    Wk = np.asarray(inputs["Wk"], dtype=np.float32)
    Wv = np.asarray(inputs["Wv"], dtype=np.float32)
    Wo = np.asarray(inputs["Wo"], dtype=np.float32)
    in_maps = []
    qTs = [np.ascontiguousarray(q[b].T.astype(bf16)) for b in range(B)]
    pTs = [np.ascontiguousarray(p[b].T.astype(bf16)) for b in range(B)]
    for core in range(NCORES):
        b, hg = core // HG, core % HG
        ds = hg * DC
        in_maps.append({
            "qT": qTs[b],
            "pT": pTs[b],
            "wq": np.ascontiguousarray(Wq[:, ds:ds + DC].astype(bf16)),
            "wk": np.ascontiguousarray(Wk[:, ds:ds + DC].astype(bf16)),
            "wv": np.ascontiguousarray(Wv[:, ds:ds + DC].astype(bf16)),
            "wo": np.ascontiguousarray(Wo[ds:ds + DC, :].astype(bf16)),
        })
    return in_maps


class _Runner:
    """Caches the compiled NEFF + jitted prep/exec/post programs.

    Per call: ship each input byte to exactly one core (sharded), then
    on-fabric allgather + slice per core (prep jit), run the bass NEFF
    (exec jit), partial-sum the 4 head-group outputs per batch on device
    (post jit), and fetch only 2 of 8 output shards.
    """

    def __init__(self):
        import jax
        import jax.numpy as jnp
        from jax import lax
        from jax.sharding import Mesh, PartitionSpec, NamedSharding
        from jax.experimental.shard_map import shard_map
        from concourse import mybir
        from concourse.bass2jax import (_bass_exec_p, fast_dispatch_compile,
                                        install_neuronx_cc_hook,
                                        partition_id_tensor)

        install_neuronx_cc_hook()
        self.jax = jax
        nc = _get_compiled()
        P = PartitionSpec

        partition_name = (nc.partition_id_tensor.name
                          if nc.partition_id_tensor else None)
        in_names, out_names, out_avals = [], [], []
        for alloc in nc.m.functions[0].allocations:
            if not isinstance(alloc, mybir.MemoryLocationSet):
                continue
            name = alloc.memorylocations[0].name
            if alloc.kind == "ExternalInput":
                if name != partition_name:
                    in_names.append(name)
            elif alloc.kind == "ExternalOutput":
                out_names.append(name)
                out_avals.append(jax.core.ShapedArray(
                    tuple(alloc.tensor_shape), mybir.dt.np(alloc.dtype)))
        all_names = list(in_names) + list(out_names)
        if partition_name is not None:
            all_names.append(partition_name)
        n_params = len(in_names)
        prep_order = ["qT", "pT", "wq", "wk", "wv", "wo"]
        self.perm = [prep_order.index(nm) for nm in in_names]

        devices = jax.devices()[:NCORES]
        mesh = Mesh(__import__("numpy").asarray(devices), ("core",))
        self.mesh = mesh
        self.shard = NamedSharding(mesh, P("core"))

        def prep_body(qT8, pT8, w38, wo8):
            core = lax.axis_index("core")
            b = core // HG
            hg = core % HG
            qT_full = lax.all_gather(qT8, "core", axis=0, tiled=True)
            pT_full = lax.all_gather(pT8, "core", axis=0, tiled=True)
            qT_b = lax.dynamic_index_in_dim(
                qT_full.reshape(B, C, T), b, keepdims=False)
            pT_b = lax.dynamic_index_in_dim(
                pT_full.reshape(B, C, T), b, keepdims=False)
            w3 = lax.all_gather(w38, "core", axis=0, tiled=True)  # [3C, C]
            ds = hg * DC
            wq_s = lax.dynamic_slice(w3, (0, ds), (C, DC))
            wk_s = lax.dynamic_slice(w3, (C, ds), (C, DC))
            wv_s = lax.dynamic_slice(w3, (2 * C, ds), (C, DC))
            wo_full = lax.all_gather(wo8, "core", axis=0, tiled=True)
            wo_s = lax.dynamic_slice(wo_full, (ds, 0), (DC, C))
            zeros = jnp.zeros((T, C), jnp.float32)
            return qT_b, pT_b, wq_s, wk_s, wv_s, wo_s, zeros

        self.prep = jax.jit(shard_map(
            prep_body, mesh=mesh,
            in_specs=(P("core"),) * 4,
            out_specs=(P("core"),) * 7, check_rep=False))

        def bass_body(*args):
            operands = list(args)
            if partition_name is not None:
                operands.append(partition_id_tensor())
            outs = _bass_exec_p.bind(
                *operands, out_avals=tuple(out_avals),
                in_names=tuple(all_names), out_names=tuple(out_names),
                lowering_input_output_aliases=(),
                sim_require_finite=True, sim_require_nnan=True, nc=nc)
            return tuple(outs)

        # Fast-dispatch (C++ dispatch path, no effect token) + no donation:
        # the bass kernel writes every byte of `out`, so the donated-zeros
        # aliasing is unnecessary; without it the same device-resident
        # argument buffers can be re-executed back-to-back, which both the
        # steady-state benchmark in test.py and repeat kernel() calls use.
        # (Output equality with the donated path was verified bit-for-bit.)
        shapes_by_name = {
            "qT": ((C, T), jnp.bfloat16), "pT": ((C, T), jnp.bfloat16),
            "wq": ((C, DC), jnp.bfloat16), "wk": ((C, DC), jnp.bfloat16),
            "wv": ((C, DC), jnp.bfloat16), "wo": ((DC, C), jnp.bfloat16),
            "out": ((T, C), jnp.float32),
        }
        arg_structs = [
            jax.ShapeDtypeStruct(
                (NCORES * shapes_by_name[nm][0][0],) + shapes_by_name[nm][0][1:],
                shapes_by_name[nm][1], sharding=self.shard)
            for nm in list(in_names) + ["out"]
        ]

        def compile_exec():
            f = jax.jit(
                shard_map(bass_body, mesh=mesh,
                          in_specs=(P("core"),) * (n_params + 1),
                          out_specs=(P("core"),) * len(out_names),
                          check_rep=False),
                keep_unused=True)
            return f.lower(*arg_structs).compile()

        self.exec = fast_dispatch_compile(compile_exec)

        groups = [[b * HG + g for g in range(HG)] for b in range(B)]

        def post_body(o):
            return lax.psum(o, "core", axis_index_groups=groups)

        self.post = jax.jit(shard_map(
            post_body, mesh=mesh, in_specs=(P("core"),),
            out_specs=P("core"), check_rep=False))

    def stage(self, inputs):
        import ml_dtypes
        jax = self.jax
        bf16 = ml_dtypes.bfloat16
        q = np.asarray(inputs["q"], dtype=np.float32)
        p = np.asarray(inputs["p"], dtype=np.float32)
        qT8 = np.concatenate(
            [np.ascontiguousarray(q[b].T.astype(bf16)) for b in range(B)],
            axis=0).reshape(NCORES, B * C // NCORES, T)
        pT8 = np.concatenate(
            [np.ascontiguousarray(p[b].T.astype(bf16)) for b in range(B)],
            axis=0).reshape(NCORES, B * C // NCORES, T)
        w38 = np.concatenate(
            [np.asarray(inputs[k], dtype=np.float32).astype(bf16)
             for k in ("Wq", "Wk", "Wv")],
            axis=0).reshape(NCORES, 3 * C // NCORES, C)
        wo8 = np.asarray(inputs["Wo"], dtype=np.float32).astype(bf16).reshape(
            NCORES, C // NCORES, C)
        return [jax.device_put(a.reshape(-1, *a.shape[2:]), self.shard)
                for a in (qT8, pT8, w38, wo8)]

    def __call__(self, inputs):
        jax = self.jax
        dev_in = self.stage(inputs)
        prep_out = self.prep(*dev_in)
        ordered = [prep_out[i] for i in self.perm] + [prep_out[6]]
        (bass_out,) = self.exec(*ordered)
        summed = self.post(bass_out)
        out = np.zeros((B, T, C), dtype=np.float32)
        shards = {s.index[0].start or 0: s.data
                  for s in summed.addressable_shards}
        for b in range(B):
            out[b] = np.asarray(shards[b * HG * T])
        return out


_RUNNER = None


def kernel(q, p, Wq, Wk, Wv, Wo):
    global _RUNNER
    inputs = dict(q=q, p=p, Wq=Wq, Wk=Wk, Wv=Wv, Wo=Wo)
    try:
        if _RUNNER is None:
            _RUNNER = _Runner()
        return _RUNNER(inputs)
    except Exception:
        import traceback
        traceback.print_exc()
        return _kernel_fallback(inputs)


def _kernel_fallback(inputs):
    from concourse.bass_utils import run_bass_kernel_spmd

    nc = _get_compiled()
    in_maps = _make_in_maps(inputs)
    res = run_bass_kernel_spmd(nc, in_maps, list(range(NCORES)))
    out = np.zeros((B, T, C), dtype=np.float32)
    for core in range(NCORES):
        out[core // HG] += res.results[core]["out"]
    return out



# revision 19
# speedup vs baseline: 265.0592x; 1.2876x over previous
"""Trainium2 Bass kernel for CrossAttention (B=2, T=S=2048, C=1024, H=16, D=64).

Sharding: 8 cores = 2 batches x 4 head-groups (tensor-parallel over heads,
4 heads per core). Each core computes its heads' attention plus the partial
output projection (Wo row slice); the host sums the 4 partials per batch.

Per-core layout trick: q/p are transposed on the host (qT/pT: [C, T]) so the
contraction dim always lands on SBUF partitions -- no on-chip transposes.

Per-core dataflow (Dc = 256 = 4 heads x 64):
  qhT[Dc,T] = Wq_s.T @ qT     (bf16 in, fp32 accum; stored f32r)
  khT[Dc,S] = Wk_s.T @ pT     (same)
  V[S,Dc]   = pT.T @ Wv_s     (same), stored bf16 with a ones column per
              head (V_aug[s, h, 64] = 1) so the AV matmul also produces the
              softmax row sums (flash-attention style).
  per head h, t-chunk (1024), s-tile (128):
      S^T[s,t] = khT_h.T @ qhT_h          (fp32r, K=64, into PSUM)
      P^T      = exp(0.125 * S^T)         (ACT, PSUM->SBUF, bf16)
      O_aug   += V_aug_h.T @ P^T          (bf16, PSUM accumulation over s)
  normalize: r = 1/rowsum (DVE), broadcast over 64 partitions (gpsimd),
             yT = O * R (DVE, written as f32r)
  out[T, C] = yT.T @ Wo_s  (fp32r) -> DMA; host sums over head groups.
"""

import numpy as np

B, T, S, C, H = 2, 2048, 2048, 1024, 16
D = C // H            # 64
NCORES = 8
HG = 4                # head groups (cores per batch)
HPC = H // HG         # heads per core = 4
DC = HPC * D          # 256 per-core head dims
KT = C // 128         # 8 contraction tiles for projections
NB = T // 512         # 4 free-dim blocks of 512
ST = S // 128         # 16 s-tiles
SCALE = 1.0 / np.sqrt(D)

_COMPILED = None


def _build():
    import concourse.bacc as bacc
    import concourse.tile as tile
    from concourse import mybir

    F32 = mybir.dt.float32
    F32R = mybir.dt.float32r
    BF16 = mybir.dt.bfloat16
    EXP = mybir.ActivationFunctionType.Exp

    nc = bacc.Bacc("TRN2", target_bir_lowering=False, debug=False,
                   num_devices=NCORES)
    qT = nc.dram_tensor("qT", [C, T], BF16, kind="ExternalInput").ap()
    pT = nc.dram_tensor("pT", [C, T], BF16, kind="ExternalInput").ap()
    wq = nc.dram_tensor("wq", [C, DC], BF16, kind="ExternalInput").ap()
    wk = nc.dram_tensor("wk", [C, DC], BF16, kind="ExternalInput").ap()
    wv = nc.dram_tensor("wv", [C, DC], BF16, kind="ExternalInput").ap()
    wo = nc.dram_tensor("wo", [DC, C], BF16, kind="ExternalInput").ap()
    out = nc.dram_tensor("out", [T, C], F32, kind="ExternalOutput").ap()

    with tile.TileContext(nc) as tc:
        with (
            tc.tile_pool(name="qx", bufs=32) as qx_pool,         # qT chunks
            tc.tile_pool(name="px", bufs=32) as px_pool,         # pT chunks
            tc.tile_pool(name="w", bufs=1) as w_pool,            # weights
            tc.tile_pool(name="proj", bufs=1) as proj_pool,      # qhT/khT/yT
            tc.tile_pool(name="vaug", bufs=1) as v_pool,
            tc.tile_pool(name="pt", bufs=8) as pt_pool,          # P^T tiles
            tc.tile_pool(name="norm", bufs=2) as norm_pool,
            tc.tile_pool(name="ostage", bufs=3) as o_pool,       # out staging
            tc.tile_pool(name="ps_pp", bufs=4, space="PSUM") as ps_pp,   # 4 banks
            tc.tile_pool(name="ps_sc", bufs=2, space="PSUM") as ps_sc,   # 4 banks
        ):
            # ---- load weights (HWDGE; f32r view of the f32 bits) ----
            wq_sb = w_pool.tile([128, KT, DC], BF16, tag="wq")
            wk_sb = w_pool.tile([128, KT, DC], BF16, tag="wk")
            wv_sb = w_pool.tile([128, KT, DC], BF16, tag="wv")
            wo_sb = w_pool.tile([128, 2, C], BF16, tag="wo")

            def emit_w_dma(w_dram, w_sb):
                nc.sync.dma_start(
                    out=w_sb[:],
                    in_=w_dram.rearrange("(k p) n -> p k n", p=128))

            # qhT/khT/yT: [2 x 128 d, T]; heads 2m,2m+1 in tile m at part 0/64
            qhT = [proj_pool.tile([128, T], BF16, tag=f"qhT{m}", name=f"qhT{m}")
                   for m in range(2)]
            khT = [proj_pool.tile([128, T], BF16, tag=f"khT{m}", name=f"khT{m}")
                   for m in range(2)]
            yT = [proj_pool.tile([128, T], BF16, tag=f"yT{m}", name=f"yT{m}")
                  for m in range(2)]
            # V_aug: [16 s-tiles][128, 4, 65] bf16; [:, h, 64] = ones
            v_sb = [v_pool.tile([128, HPC, D + 1], BF16, tag=f"v{st}",
                                name=f"v{st}")
                    for st in range(ST)]
            for st in range(ST):
                nc.vector.memset(v_sb[st][:, :, D:D + 1], 1.0)

            def emit_px_dma(nb):
                px_c = [px_pool.tile([128, 512], BF16, tag="px",
                                     name=f"px{kt}_{nb}") for kt in range(KT)]
                for kt in range(KT):
                    nc.sync.dma_start(
                        out=px_c[kt][:],
                        in_=pT[kt * 128:(kt + 1) * 128,
                               nb * 512:(nb + 1) * 512])
                return px_c

            def emit_qx_dma(nb):
                qx_c = [qx_pool.tile([128, 512], BF16, tag="qx",
                                     name=f"qx{kt}_{nb}") for kt in range(KT)]
                for kt in range(KT):
                    nc.sync.dma_start(
                        out=qx_c[kt][:],
                        in_=qT[kt * 128:(kt + 1) * 128,
                               nb * 512:(nb + 1) * 512])
                return qx_c

            def emit_khT(nb, px_c, evict_act=False):
                for mt in range(2):
                    ps = ps_pp.tile([128, 512], F32, tag="pp")
                    for kt in range(KT):
                        nc.tensor.matmul(
                            ps[:],
                            wk_sb[:, kt, mt * 128:(mt + 1) * 128],
                            px_c[kt][:],
                            start=(kt == 0), stop=(kt == KT - 1))
                    dst = khT[mt][:, nb * 512:(nb + 1) * 512]
                    if evict_act:
                        nc.scalar.copy(dst, ps[:])
                    else:
                        nc.vector.tensor_copy(dst, ps[:])

            def emit_qhT(nb, qx_c):
                for mt in range(2):
                    ps = ps_pp.tile([128, 512], F32, tag="pp")
                    for kt in range(KT):
                        nc.tensor.matmul(
                            ps[:],
                            wq_sb[:, kt, mt * 128:(mt + 1) * 128],
                            qx_c[kt][:],
                            start=(kt == 0), stop=(kt == KT - 1))
                    nc.vector.tensor_copy(
                        qhT[mt][:, nb * 512:(nb + 1) * 512], ps[:])

            def emit_V(nb, px_c, evict_act=False):
                for j in range(4):
                    st = nb * 4 + j
                    ps = ps_pp.tile([128, DC], F32, tag="pp")
                    for kt in range(KT):
                        nc.tensor.matmul(
                            ps[:], px_c[kt][:, j * 128:(j + 1) * 128],
                            wv_sb[:, kt, :],
                            start=(kt == 0), stop=(kt == KT - 1))
                    dst = v_sb[st][:, :, 0:D]
                    if evict_act:
                        nc.scalar.copy(dst, ps[:])
                    else:
                        nc.vector.tensor_copy(dst, ps[:])

            def emit_scores_quad(mt, ch, st):
                """Scores + exp for BOTH heads of tile mt (partitions 0-63 /
                64-127) at s-tile st.  The four matmuls alternate PE row
                groups (tile_position rows 0 vs 64, inferred from the
                operand base partitions), so consecutive matmuls run
                CONCURRENTLY in the array and each LDWEIGHTS overlaps the
                other row group's in-flight matmul."""
                t0 = ch * 1024
                scA = ps_sc.tile([128, 1024], F32, tag="scA", bufs=1)
                scB = ps_sc.tile([128, 1024], F32, tag="scB", bufs=1)
                for j in range(2):
                    for hh in range(2):
                        sc = scA if hh == 0 else scB
                        po = hh * 64
                        nc.tensor.matmul(
                            sc[:, j * 512:(j + 1) * 512],
                            khT[mt][po:po + 64, st * 128:(st + 1) * 128],
                            qhT[mt][po:po + 64,
                                    t0 + j * 512:t0 + (j + 1) * 512],
                            start=True, stop=True)
                ptA = pt_pool.tile([128, 1024], BF16, tag="ptA", bufs=6)
                nc.scalar.activation(ptA[:], scA[:], EXP, scale=float(SCALE))
                ptB = pt_pool.tile([128, 1024], BF16, tag="ptB", bufs=16)
                nc.scalar.activation(ptB[:], scB[:], EXP, scale=float(SCALE))
                return ptA, ptB

            def emit_av(h, st, ptile, o_ps):
                for j in range(2):
                    nc.tensor.matmul(
                        o_ps[j][0:D + 1, :],
                        v_sb[st][:, h, :],
                        ptile[:, j * 512:(j + 1) * 512],
                        start=(st == 0), stop=(st == ST - 1))

            def emit_attn_pair(mt, ch, hooks=None, lag=0):
                """Attention for heads (2mt, 2mt+1) over t-chunk ch.  Head
                A's AV trails the score quads by `lag` s-tiles; head B's
                P^T tiles are stashed in SBUF and its AV replays as a
                PE-only burst at the end (PSUM pressure: only one o_ps
                pair is live during the quad loop, leaving banks for the
                interleaved projection work in `hooks`)."""
                hA, hB = 2 * mt, 2 * mt + 1
                o_psA = [ps_pp.tile([128, 512], F32, tag="pp",
                                    name=f"oA{mt}_{ch}_{j}") for j in range(2)]
                ptB_stash = {}
                for step in range(ST + lag):
                    if step < ST:
                        for fn_ in (hooks or {}).get(step, []):
                            fn_()
                        ptA, ptB_stash[step] = emit_scores_quad(mt, ch, step)
                        if lag == 0:
                            emit_av(hA, step, ptA, o_psA)
                        else:
                            ptB_stash[(step, 'A')] = ptA
                    if lag and step >= lag:
                        emit_av(hA, step - lag,
                                ptB_stash.pop((step - lag, 'A')), o_psA)
                emit_norm(hA, ch, o_psA)
                o_psB = [ps_pp.tile([128, 512], F32, tag="pp",
                                    name=f"oB{mt}_{ch}_{j}") for j in range(2)]
                for st in range(ST):
                    emit_av(hB, st, ptB_stash.pop(st), o_psB)
                emit_norm(hB, ch, o_psB)

            def emit_norm(h, ch, o_ps):
                # per-j chain so each o_ps bank frees as early as possible
                mt, po = h // 2, (h % 2) * 64
                t0 = ch * 1024
                for j in range(2):
                    r_sb = norm_pool.tile([1, 512], F32, tag="r",
                                          name=f"r{h}_{ch}_{j}")
                    nc.vector.reciprocal(r_sb[:], o_ps[j][D:D + 1, :])
                    R_sb = norm_pool.tile([64, 512], F32, tag="R",
                                          name=f"R{h}_{ch}_{j}")
                    nc.gpsimd.partition_broadcast(R_sb[:], r_sb[:])
                    nc.vector.tensor_mul(
                        yT[mt][po:po + 64, t0 + j * 512:t0 + (j + 1) * 512],
                        o_ps[j][0:D, :],
                        R_sb[:])

            def emit_outproj(mt2_lo, mt2_hi, act_evict=False):
                for mt2 in range(mt2_lo, mt2_hi):
                    ost = o_pool.tile([128, C], F32, tag="ost",
                                      name=f"ost{mt2}")
                    for nb2 in range(2):
                        ps = ps_pp.tile([128, 512], F32, tag="pp")
                        for kt2 in range(2):
                            nc.tensor.matmul(
                                ps[:],
                                yT[kt2][:, mt2 * 128:(mt2 + 1) * 128],
                                wo_sb[:, kt2, nb2 * 512:(nb2 + 1) * 512],
                                start=(kt2 == 0), stop=(kt2 == 1))
                        dst = ost[:, nb2 * 512:(nb2 + 1) * 512]
                        if act_evict and mt2 % 2 == 0:
                            nc.scalar.copy(dst, ps[:])
                        else:
                            nc.vector.tensor_copy(dst, ps[:])
                        if act_evict:
                            nc.sync.dma_start(
                                out=out[mt2 * 128:(mt2 + 1) * 128,
                                        nb2 * 512:(nb2 + 1) * 512],
                                in_=dst)
                    if not act_evict:
                        nc.sync.dma_start(
                            out=out[mt2 * 128:(mt2 + 1) * 128, :], in_=ost[:])

            # ---- interleaved emission: overlap DMA / proj / attention /
            # outproj. DMA FIFO order tracks the critical chain to the
            # first exp: wk -> px0 -> qx0/qx1 -> khT0 -> qhT0/1 -> scores.
            warm = norm_pool.tile([1, 8], F32, tag="warm")
            nc.vector.memset(warm[:], 0.0)
            warm2 = norm_pool.tile([1, 8], F32, tag="warm2")
            nc.scalar.activation(warm2[:], warm[:], EXP)  # preload exp table

            emit_w_dma(wk, wk_sb)
            emit_w_dma(wq, wq_sb)
            emit_w_dma(wv, wv_sb)
            nc.sync.dma_start(
                out=wo_sb[:],
                in_=wo.rearrange("(k p) n -> p k n", p=128))
            px0 = emit_px_dma(0)
            qx0 = emit_qx_dma(0)
            qx1 = emit_qx_dma(1)
            px1 = emit_px_dma(1)
            emit_khT(0, px0)
            emit_qhT(0, qx0)
            emit_qhT(1, qx1)
            # first pair-block (heads 0+1, t 0-1023) with the remaining
            # khT/V projection work and px DMAs interleaved at the s-tile
            # steps where their results are first needed; head A's AV lags
            # the quads by 4 s-tiles so emit_V(nb) can land just ahead of
            # the first AV that reads it.
            px23 = {}
            hooks = {
                2: [lambda: emit_khT(1, px1)],
                4: [lambda: emit_V(0, px0)],
                6: [lambda: px23.setdefault(2, emit_px_dma(2)),
                    lambda: emit_khT(2, px23[2])],
                8: [lambda: emit_V(1, px1)],
                10: [lambda: px23.setdefault(3, emit_px_dma(3)),
                     lambda: emit_khT(3, px23[3])],
                11: [lambda: emit_V(2, px23[2])],
                13: [lambda: emit_V(3, px23[3])],
            }
            emit_attn_pair(0, 0, hooks=hooks, lag=4)
            # qhT for the second t-chunk, deferred out of the DMA-bound front
            qx2 = emit_qx_dma(2)
            qx3 = emit_qx_dma(3)
            emit_qhT(2, qx2)
            emit_qhT(3, qx3)
            emit_attn_pair(1, 0)
            emit_attn_pair(0, 1)
            emit_outproj(0, 8)
            emit_attn_pair(1, 1)
            emit_outproj(8, 16, act_evict=True)

    nc.compile()
    return nc


def _get_compiled():
    global _COMPILED
    if _COMPILED is None:
        _COMPILED = _build()
    return _COMPILED


def _make_in_maps(inputs):
    import ml_dtypes
    bf16 = ml_dtypes.bfloat16
    q = np.asarray(inputs["q"], dtype=np.float32)
    p = np.asarray(inputs["p"], dtype=np.float32)
    Wq = np.asarray(inputs["Wq"], dtype=np.float32)
    Wk = np.asarray(inputs["Wk"], dtype=np.float32)
    Wv = np.asarray(inputs["Wv"], dtype=np.float32)
    Wo = np.asarray(inputs["Wo"], dtype=np.float32)
    in_maps = []
    qTs = [np.ascontiguousarray(q[b].T.astype(bf16)) for b in range(B)]
    pTs = [np.ascontiguousarray(p[b].T.astype(bf16)) for b in range(B)]
    for core in range(NCORES):
        b, hg = core // HG, core % HG
        ds = hg * DC
        in_maps.append({
            "qT": qTs[b],
            "pT": pTs[b],
            "wq": np.ascontiguousarray(Wq[:, ds:ds + DC].astype(bf16)),
            "wk": np.ascontiguousarray(Wk[:, ds:ds + DC].astype(bf16)),
            "wv": np.ascontiguousarray(Wv[:, ds:ds + DC].astype(bf16)),
            "wo": np.ascontiguousarray(Wo[ds:ds + DC, :].astype(bf16)),
        })
    return in_maps


class _Runner:
    """Caches the compiled NEFF + jitted prep/exec/post programs.

    Per call: ship each input byte to exactly one core (sharded), then
    on-fabric allgather + slice per core (prep jit), run the bass NEFF
    (exec jit), partial-sum the 4 head-group outputs per batch on device
    (post jit), and fetch only 2 of 8 output shards.
    """

    def __init__(self):
        import jax
        import jax.numpy as jnp
        from jax import lax
        from jax.sharding import Mesh, PartitionSpec, NamedSharding
        from jax.experimental.shard_map import shard_map
        from concourse import mybir
        from concourse.bass2jax import (_bass_exec_p, fast_dispatch_compile,
                                        install_neuronx_cc_hook,
                                        partition_id_tensor)

        install_neuronx_cc_hook()
        self.jax = jax
        nc = _get_compiled()
        P = PartitionSpec

        partition_name = (nc.partition_id_tensor.name
                          if nc.partition_id_tensor else None)
        in_names, out_names, out_avals = [], [], []
        for alloc in nc.m.functions[0].allocations:
            if not isinstance(alloc, mybir.MemoryLocationSet):
                continue
            name = alloc.memorylocations[0].name
            if alloc.kind == "ExternalInput":
                if name != partition_name:
                    in_names.append(name)
            elif alloc.kind == "ExternalOutput":
                out_names.append(name)
                out_avals.append(jax.core.ShapedArray(
                    tuple(alloc.tensor_shape), mybir.dt.np(alloc.dtype)))
        all_names = list(in_names) + list(out_names)
        if partition_name is not None:
            all_names.append(partition_name)
        n_params = len(in_names)
        prep_order = ["qT", "pT", "wq", "wk", "wv", "wo"]
        self.perm = [prep_order.index(nm) for nm in in_names]

        devices = jax.devices()[:NCORES]
        mesh = Mesh(__import__("numpy").asarray(devices), ("core",))
        self.mesh = mesh
        self.shard = NamedSharding(mesh, P("core"))

        def prep_body(qT8, pT8, w38, wo8):
            core = lax.axis_index("core")
            b = core // HG
            hg = core % HG
            qT_full = lax.all_gather(qT8, "core", axis=0, tiled=True)
            pT_full = lax.all_gather(pT8, "core", axis=0, tiled=True)
            qT_b = lax.dynamic_index_in_dim(
                qT_full.reshape(B, C, T), b, keepdims=False)
            pT_b = lax.dynamic_index_in_dim(
                pT_full.reshape(B, C, T), b, keepdims=False)
            w3 = lax.all_gather(w38, "core", axis=0, tiled=True)  # [3C, C]
            ds = hg * DC
            wq_s = lax.dynamic_slice(w3, (0, ds), (C, DC))
            wk_s = lax.dynamic_slice(w3, (C, ds), (C, DC))
            wv_s = lax.dynamic_slice(w3, (2 * C, ds), (C, DC))
            wo_full = lax.all_gather(wo8, "core", axis=0, tiled=True)
            wo_s = lax.dynamic_slice(wo_full, (ds, 0), (DC, C))
            zeros = jnp.zeros((T, C), jnp.float32)
            return qT_b, pT_b, wq_s, wk_s, wv_s, wo_s, zeros

        self.prep = jax.jit(shard_map(
            prep_body, mesh=mesh,
            in_specs=(P("core"),) * 4,
            out_specs=(P("core"),) * 7, check_rep=False))

        def bass_body(*args):
            operands = list(args)
            if partition_name is not None:
                operands.append(partition_id_tensor())
            outs = _bass_exec_p.bind(
                *operands, out_avals=tuple(out_avals),
                in_names=tuple(all_names), out_names=tuple(out_names),
                lowering_input_output_aliases=(),
                sim_require_finite=True, sim_require_nnan=True, nc=nc)
            return tuple(outs)

        # Fast-dispatch (C++ dispatch path, no effect token) + no donation:
        # the bass kernel writes every byte of `out`, so the donated-zeros
        # aliasing is unnecessary; without it the same device-resident
        # argument buffers can be re-executed back-to-back, which both the
        # steady-state benchmark in test.py and repeat kernel() calls use.
        # (Output equality with the donated path was verified bit-for-bit.)
        shapes_by_name = {
            "qT": ((C, T), jnp.bfloat16), "pT": ((C, T), jnp.bfloat16),
            "wq": ((C, DC), jnp.bfloat16), "wk": ((C, DC), jnp.bfloat16),
            "wv": ((C, DC), jnp.bfloat16), "wo": ((DC, C), jnp.bfloat16),
            "out": ((T, C), jnp.float32),
        }
        arg_structs = [
            jax.ShapeDtypeStruct(
                (NCORES * shapes_by_name[nm][0][0],) + shapes_by_name[nm][0][1:],
                shapes_by_name[nm][1], sharding=self.shard)
            for nm in list(in_names) + ["out"]
        ]

        def compile_exec():
            f = jax.jit(
                shard_map(bass_body, mesh=mesh,
                          in_specs=(P("core"),) * (n_params + 1),
                          out_specs=(P("core"),) * len(out_names),
                          check_rep=False),
                keep_unused=True)
            return f.lower(*arg_structs).compile()

        self.exec = fast_dispatch_compile(compile_exec)

        groups = [[b * HG + g for g in range(HG)] for b in range(B)]

        def post_body(o):
            return lax.psum(o, "core", axis_index_groups=groups)

        self.post = jax.jit(shard_map(
            post_body, mesh=mesh, in_specs=(P("core"),),
            out_specs=P("core"), check_rep=False))

    def stage(self, inputs):
        import ml_dtypes
        jax = self.jax
        bf16 = ml_dtypes.bfloat16
        q = np.asarray(inputs["q"], dtype=np.float32)
        p = np.asarray(inputs["p"], dtype=np.float32)
        qT8 = np.concatenate(
            [np.ascontiguousarray(q[b].T.astype(bf16)) for b in range(B)],
            axis=0).reshape(NCORES, B * C // NCORES, T)
        pT8 = np.concatenate(
            [np.ascontiguousarray(p[b].T.astype(bf16)) for b in range(B)],
            axis=0).reshape(NCORES, B * C // NCORES, T)
        w38 = np.concatenate(
            [np.asarray(inputs[k], dtype=np.float32).astype(bf16)
             for k in ("Wq", "Wk", "Wv")],
            axis=0).reshape(NCORES, 3 * C // NCORES, C)
        wo8 = np.asarray(inputs["Wo"], dtype=np.float32).astype(bf16).reshape(
            NCORES, C // NCORES, C)
        return [jax.device_put(a.reshape(-1, *a.shape[2:]), self.shard)
                for a in (qT8, pT8, w38, wo8)]

    def __call__(self, inputs):
        jax = self.jax
        dev_in = self.stage(inputs)
        prep_out = self.prep(*dev_in)
        ordered = [prep_out[i] for i in self.perm] + [prep_out[6]]
        (bass_out,) = self.exec(*ordered)
        summed = self.post(bass_out)
        out = np.zeros((B, T, C), dtype=np.float32)
        shards = {s.index[0].start or 0: s.data
                  for s in summed.addressable_shards}
        for b in range(B):
            out[b] = np.asarray(shards[b * HG * T])
        return out


_RUNNER = None


def kernel(q, p, Wq, Wk, Wv, Wo):
    global _RUNNER
    inputs = dict(q=q, p=p, Wq=Wq, Wk=Wk, Wv=Wv, Wo=Wo)
    try:
        if _RUNNER is None:
            _RUNNER = _Runner()
        return _RUNNER(inputs)
    except Exception:
        import traceback
        traceback.print_exc()
        return _kernel_fallback(inputs)


def _kernel_fallback(inputs):
    from concourse.bass_utils import run_bass_kernel_spmd

    nc = _get_compiled()
    in_maps = _make_in_maps(inputs)
    res = run_bass_kernel_spmd(nc, in_maps, list(range(NCORES)))
    out = np.zeros((B, T, C), dtype=np.float32)
    for core in range(NCORES):
        out[core // HG] += res.results[core]["out"]
    return out

